# revision 1
# baseline (speedup 1.0000x reference)
"""Contrastive loss (SimCLR-style, masked-diagonal logsumexp) on 8 Trainium2
NeuronCores via Bass/Tile.

Math (matches the jax reference):
    a = anchor / ||anchor||_row ; p = positive / ||positive||_row
    F = concat([a, p])                         # [R=2B, D]
    sim = (F F^T) / T with diagonal masked to -inf
    lse_i = log(sum_j exp(sim_ij))
    pos_i = <a_i, p_i> / T  (duplicated for both halves)
    loss = sum_i (lse_i - pos_i) * lab_i / max(sum_i lab_i, 1)

Distribution: data-parallel over the row dim of F. Core c owns rows
[c*Rc, (c+1)*Rc). Each core computes its [Rc, R] block of the similarity
matrix as raw-Gram bf16 matmuls (lhsT = its column-slice of F^T, rhs = full
F^T streamed), then applies the normalization on the fly:

    exp(sim_ij) = exp((inv_i/T) * (G_ij * inv_j))

where inv = 1/||f|| is computed on-device from f32 natural-layout shards and
exchanged with one tiny AllGather (32KB). inv_j is applied by one DVE
tensor_tensor against a partition-broadcast [128, R] tile; inv_i/T rides the
ACT Exp's per-partition scale; the row-sum rides the ACT's accum_out. The
diagonal is masked by zeroing inv_j at position (p, diag_col(p)) with a
per-core 0/1 host mask, which makes exp contribute exactly 1.0 there; the
final logsumexp uses ln(rowsum - 1) via the ACT bias to remove it.

Final reduction: per-core (sum_i per_row, sum_i lab) -> PE ones-matmul over
partitions -> scalar AllReduce(add) -> loss = s * exp(-ln(max(n,1))) (the
Reciprocal ACT table is banned for accuracy; exp/ln stay within one ACT
table set).
"""

import os
import numpy as np
import ml_dtypes


# ---------------------------------------------------------------- config ----
class CFG:
    B = 4096
    D = 1024
    NC = 8           # cores
    JW = 512         # j-chunk width (one PSUM bank of f32)
    TEMP = 0.07

    @property
    def R(self):
        return 2 * self.B           # total rows of F

    @property
    def RC(self):
        return self.R // self.NC    # rows per core

    @property
    def MT(self):
        return self.RC // 128       # m-tiles per core

    @property
    def KC(self):
        return self.D // 128        # k-chunks

    @property
    def JC(self):
        return self.R // self.JW    # j-chunks


_BUILD_CACHE = {}


# ----------------------------------------------------------------- build ----
def build_nc(cfg: CFG, stage: str = "full"):
    """Emit the single SPMD program (identical instruction stream on all
    cores; every per-core difference comes in through input tensors).

    Local row naming: core-local row r = p*MT + m (p = SBUF partition,
    m = m-tile). G m-tile mt therefore holds rows {q*MT + mt} at output
    partition q, and the flattened [128, MT] stat tiles are row-ordered,
    which makes the inv DRAM round-trips contiguous.

    j-axis naming: slot s = host-rolled j-chunk; slot s is global chunk
    (OWN*c + s) % JC, so slots 0..OWN-1 are this core's own columns (inv
    known locally before the AllGather, diagonal mask lives only there).
    """
    from contextlib import ExitStack

    import concourse.bass as bass
    import concourse.tile as tile
    from concourse import bacc, mybir

    f32 = mybir.dt.float32
    bf16 = mybir.dt.bfloat16
    i32 = mybir.dt.int32
    Act = mybir.ActivationFunctionType
    Alu = mybir.AluOpType

    D, R, RC, MT, KC, JC, JW = cfg.D, cfg.R, cfg.RC, cfg.MT, cfg.KC, cfg.JC, cfg.JW
    OWN = RC // JW
    invT = 1.0 / cfg.TEMP

    nc = bacc.Bacc("TRN2", target_bir_lowering=False, debug=False,
                   num_devices=cfg.NC)

    ft_c = nc.dram_tensor("ft_c", [JC, 128, KC, JW], bf16,
                          kind="ExternalInput").ap()
    lhs_p = nc.dram_tensor("lhs_p", [128, KC, RC], bf16,
                           kind="ExternalInput").ap()
    feat_p = nc.dram_tensor("feat_p", [128, MT, D], bf16,
                            kind="ExternalInput").ap()
    mate_p = nc.dram_tensor("mate_p", [128, MT, D], bf16,
                            kind="ExternalInput").ap()
    labf = nc.dram_tensor("labf", [RC], f32, kind="ExternalInput").ap()
    maskb = nc.dram_tensor("maskb", [128, RC], f32, kind="ExternalInput").ap()
    cidx = nc.dram_tensor("cidx", [JC], i32, kind="ExternalInput").ap()
    loss = nc.dram_tensor("loss", [1, 1], f32, kind="ExternalOutput").ap()

    groups = [list(range(cfg.NC))]

    with tile.TileContext(nc) as tc, ExitStack() as ctx:
        const = ctx.enter_context(tc.tile_pool(name="const", bufs=1))
        rhsp = ctx.enter_context(tc.tile_pool(name="rhs", bufs=3))
        yp = ctx.enter_context(tc.tile_pool(name="y", bufs=4))
        scr = ctx.enter_context(tc.tile_pool(name="scr", bufs=2))
        esp = ctx.enter_context(tc.tile_pool(name="es", bufs=2))
        smal = ctx.enter_context(tc.tile_pool(name="small", bufs=1))
        gp = ctx.enter_context(tc.tile_pool(name="g", bufs=7, space="PSUM"))
        fp = ctx.enter_context(tc.tile_pool(name="fpsum", bufs=1, space="PSUM"))
        dram = ctx.enter_context(tc.tile_pool(name="dram", bufs=1, space="DRAM"))

        # ---- staging -------------------------------------------------------
        # sync ring: matmul stream only. Per-k splits let the first k-loop
        # start as soon as its first 256KB lands instead of after 3MB.
        lhsT = const.tile([128, KC, RC], bf16)
        for k in range(KC):
            nc.sync.dma_start(lhsT[:, k, :], lhs_p[:, k, :])
        feat_sb = const.tile([128, MT, D], bf16)
        for m in range(MT):
            nc.gpsimd.dma_start(feat_sb[:, m, :], feat_p[:, m, :])
        maskB = const.tile([128, RC], f32)
        nc.scalar.dma_start(maskB[:], maskb)
        lab_sb = smal.tile([128, MT], f32)
        nc.scalar.dma_start(lab_sb[:], labf.rearrange("(p m) -> p m", m=MT))
        cidx_sb = smal.tile([JC, 1], i32)
        nc.scalar.dma_start(cidx_sb[:], cidx.rearrange("(a b) -> a b", b=1))
        ones = smal.tile([128, 1], f32)
        nc.vector.memset(ones[:], 1.0)
        negone = smal.tile([128, 1], f32)
        nc.vector.memset(negone[:], -1.0)

        # ---- own-row norms -> inv -> AllGather (the critical path) ---------
        nsq_f = smal.tile([128, MT], f32)
        for m in range(MT):
            s1 = scr.tile([128, D], f32, tag="sq")
            nc.scalar.activation(s1[:], feat_sb[:, m, :], Act.Square,
                                 accum_out=nsq_f[:, m:m + 1])
        lnf = smal.tile([128, MT], f32)
        nc.scalar.activation(lnf[:], nsq_f[:], Act.Ln)
        inv_f = smal.tile([128, MT], f32)
        nc.scalar.activation(inv_f[:], lnf[:], Act.Exp, scale=-0.5)
        invOT = smal.tile([128, MT], f32)
        nc.vector.tensor_scalar_mul(invOT[:], inv_f[:], float(invT))

        # contiguous row-order spill of inv (row r = p*MT + m)
        inv_shard = dram.tile([RC], f32)
        nc.gpsimd.dma_start(inv_shard[:].rearrange("(p m) -> p m", m=MT),
                            inv_f[:])
        inv_all = dram.tile([cfg.NC, RC], f32)
        nc.gpsimd.collective_compute(
            "AllGather", Alu.bypass, replica_groups=groups,
            ins=[inv_shard[:].opt()], outs=[inv_all[:].opt()])

        # own-column invB slots straight from the local inv (pre-AllGather);
        # the diagonal mask lives entirely inside these slots.
        invB = const.tile([128, R], f32)
        for s in range(OWN):
            sl = slice(s * JW, (s + 1) * JW)
            nc.gpsimd.dma_start(invB[:, sl],
                                inv_shard[:][sl].partition_broadcast(128))
            nc.vector.tensor_mul(invB[:, sl], invB[:, sl], maskB[:, sl])

        # mate shard rides the gpsimd ring while the AllGather is in flight
        mate_sb = const.tile([128, MT, D], bf16)
        for m in range(MT):
            nc.gpsimd.dma_start(mate_sb[:, m, :], mate_p[:, m, :])

        # roll the gathered inv into slot order (host int32 chunk indices),
        # bounce via DRAM, then per-slot partition-broadcasts (no DVE work).
        rolled_sb = smal.tile([JC, JW], f32)
        nc.gpsimd.indirect_dma_start(
            out=rolled_sb[:], out_offset=None,
            in_=inv_all[:].rearrange("a (c w) -> (a c) w", w=JW),
            in_offset=bass.IndirectOffsetOnAxis(ap=cidx_sb[:, 0:1], axis=0))
        rolled_dram = dram.tile([JC, JW], f32)
        nc.gpsimd.dma_start(rolled_dram[:], rolled_sb[:])
        for s in range(OWN, JC):
            sl = slice(s * JW, (s + 1) * JW)
            nc.gpsimd.dma_start(invB[:, sl],
                                rolled_dram[s].partition_broadcast(128))

        # ---- main loop: G block-matmul + fused softmax-denominator ---------
        # mate-norm / positive-pair reductions are sprinkled between early
        # slots so they fill DVE slack instead of blocking the first
        # epilogues or piling up in the tail.
        nsq_m = smal.tile([128, MT], f32)
        crossS = smal.tile([128, MT], f32)

        def mate_piece(m):
            s2 = scr.tile([128, D], f32, tag="sq")
            nc.vector.scalar_tensor_tensor(
                out=s2[:], in0=mate_sb[:, m, :], scalar=1.0,
                in1=mate_sb[:, m, :], op0=Alu.mult, op1=Alu.mult,
                accum_out=nsq_m[:, m:m + 1])
            s3 = scr.tile([128, D], f32, tag="sq")
            nc.vector.scalar_tensor_tensor(
                out=s3[:], in0=feat_sb[:, m, :], scalar=1.0,
                in1=mate_sb[:, m, :], op0=Alu.mult, op1=Alu.mult,
                accum_out=crossS[:, m:m + 1])

        rs_all = smal.tile([128, MT, JC], f32)
        for s in range(JC):
            rhs = rhsp.tile([128, KC, JW], bf16)
            if s == 0:
                for k in range(KC):
                    nc.sync.dma_start(rhs[:, k, :], ft_c[s, :, k, :])
            else:
                nc.sync.dma_start(rhs[:], ft_c[s])
            for mt in range(MT):
                g = gp.tile([128, JW], f32)
                for k in range(KC):
                    nc.tensor.matmul(
                        g[:], lhsT[:, k, mt * 128:(mt + 1) * 128],
                        rhs[:, k, :], start=(k == 0), stop=(k == KC - 1))
                y = yp.tile([128, JW], f32)
                nc.vector.tensor_mul(y[:], g[:], invB[:, s * JW:(s + 1) * JW])
                es = esp.tile([128, JW], f32)
                nc.scalar.activation(es[:], y[:], Act.Exp,
                                     scale=invOT[:, mt:mt + 1],
                                     accum_out=rs_all[:, mt, s:s + 1])
            if OWN <= s < OWN + MT:
                mate_piece(s - OWN)
            if s == OWN + MT:
                lnm = smal.tile([128, MT], f32)
                nc.scalar.activation(lnm[:], nsq_m[:], Act.Ln)
                inv_m = smal.tile([128, MT], f32)
                nc.scalar.activation(inv_m[:], lnm[:], Act.Exp, scale=-0.5)
                cf = smal.tile([128, MT], f32)
                nc.vector.tensor_mul(cf[:], inv_f[:], inv_m[:])
                crossT = smal.tile([128, MT], f32)
                nc.vector.tensor_mul(crossT[:], crossS[:], cf[:])
                nc.vector.tensor_scalar_mul(crossT[:], crossT[:], float(invT))

        # ---- per-row tail --------------------------------------------------
        rsum = smal.tile([128, MT], f32)
        nc.vector.tensor_reduce(rsum[:], rs_all[:],
                                axis=mybir.AxisListType.X, op=Alu.add)
        # lse = ln(rowsum - 1): the masked diagonal contributed exp(0) = 1
        lse = smal.tile([128, MT], f32)
        nc.scalar.activation(lse[:], rsum[:], Act.Ln, bias=negone[:])
        diff = smal.tile([128, MT], f32)
        nc.vector.tensor_sub(diff[:], lse[:], crossT[:])
        pn = smal.tile([128, 2], f32)
        pscr = smal.tile([128, MT], f32)
        nc.vector.scalar_tensor_tensor(
            out=pscr[:], in0=diff[:], scalar=1.0, in1=lab_sb[:],
            op0=Alu.mult, op1=Alu.mult, accum_out=pn[:, 0:1])
        nc.vector.tensor_reduce(pn[:, 1:2], lab_sb[:],
                                axis=mybir.AxisListType.X, op=Alu.add)

        # partition-reduce both sums with one tiny matmul: pn^T @ ones
        pr = fp.tile([2, 1], f32)
        nc.tensor.matmul(pr[:], pn[:], ones[:], start=True, stop=True)

        prs = smal.tile([2, 1], f32)
        nc.vector.tensor_copy(prs[:], pr[:])
        ar_in = dram.tile([2, 1], f32)
        nc.sync.dma_start(ar_in[:], prs[:])
        ar_out = dram.tile([2, 1], f32)
        nc.gpsimd.collective_compute(
            "AllReduce", Alu.add, replica_groups=groups,
            ins=[ar_in[:].opt()], outs=[ar_out[:].opt()])

        fin = smal.tile([1, 2], f32)
        nc.sync.dma_start(fin[:], ar_out[:].rearrange("a b -> b a"))
        n1 = smal.tile([1, 1], f32)
        nc.vector.tensor_scalar_max(n1[:], fin[:, 1:2], 1.0)
        lnn = smal.tile([1, 1], f32)
        nc.scalar.activation(lnn[:], n1[:], Act.Ln)
        invn = smal.tile([1, 1], f32)
        nc.scalar.activation(invn[:], lnn[:], Act.Exp, scale=-1.0)
        lv = smal.tile([1, 1], f32)
        nc.vector.tensor_mul(lv[:], fin[:, 0:1], invn[:])
        nc.sync.dma_start(loss, lv[:])

    nc.finalize()
    return nc


# ------------------------------------------------------------ host side -----
def make_in_maps(cfg: CFG, anchor, positive, labels):
    a = np.asarray(anchor, dtype=np.float32)
    p = np.asarray(positive, dtype=np.float32)
    lab = np.asarray(labels).astype(np.float32)
    B, D, NC, RC, MT, R = cfg.B, cfg.D, cfg.NC, cfg.RC, cfg.MT, cfg.R
    half = NC // 2
    feats = np.concatenate([a, p], axis=0)                  # [R, D]
    ft_bf = np.ascontiguousarray(feats.T).astype(ml_dtypes.bfloat16)

    KC, JC, JW = cfg.KC, cfg.JC, cfg.JW
    OWN = RC // JW
    # ft_c[jc, p, k, n] = ftT[k*128+p, jc*JW+n]
    ft_c = np.ascontiguousarray(
        ft_bf.reshape(KC, 128, JC, JW).transpose(2, 1, 0, 3))

    # lhsT column order inside m-tile mt is q -> local row q*MT + mt
    colperm = np.add.outer(np.arange(MT), np.arange(128) * MT).reshape(-1)

    idx = np.arange(128)
    in_maps = []
    for c in range(NC):
        lr = (c % half) * RC
        # slot s of this core's j-axis = global chunk (OWN*c + s) % JC
        gperm = (OWN * c + np.arange(JC)) % JC
        # diagonal zeros live in the own slots: col (local row) q*MT + mt
        # for partition q of m-tile mt
        maskB = np.ones((128, RC), np.float32)
        for mt in range(MT):
            maskB[idx, idx * MT + mt] = 0.0
        if c < half:
            fn, mn = a[lr:lr + RC], p[lr:lr + RC]
        else:
            fn, mn = p[lr:lr + RC], a[lr:lr + RC]
        lhs_p = np.ascontiguousarray(
            ft_bf[:, c * RC:(c + 1) * RC][:, colperm]
            .reshape(KC, 128, RC).transpose(1, 0, 2))
        in_maps.append({
            "ft_c": np.ascontiguousarray(ft_c[gperm]),
            "lhs_p": lhs_p,
            "feat_p": np.ascontiguousarray(
                fn.reshape(128, MT, D).astype(ml_dtypes.bfloat16)),
            "mate_p": np.ascontiguousarray(
                mn.reshape(128, MT, D).astype(ml_dtypes.bfloat16)),
            "labf": np.ascontiguousarray(lab[lr:lr + RC]),
            "maskb": maskB,
            "cidx": gperm.astype(np.int32),
        })
    return in_maps


LAST_RESULTS = None


def kernel(anchor_features, positive_features, labels):
    global LAST_RESULTS
    from concourse.bass_utils import run_bass_kernel_spmd

    cfg = CFG()
    key = (cfg.B, cfg.D, cfg.NC)
    if key not in _BUILD_CACHE:
        _BUILD_CACHE[key] = build_nc(cfg)
    nc = _BUILD_CACHE[key]

    in_maps = make_in_maps(cfg, anchor_features, positive_features, labels)
    trace = bool(int(os.environ.get("KERNEL_TRACE", "0")))
    res = run_bass_kernel_spmd(nc, in_maps, list(range(cfg.NC)), trace=trace)
    LAST_RESULTS = res
    out = np.asarray(res.results[0]["loss"], dtype=np.float32)
    return out.reshape(())



# revision 6
# speedup vs baseline: 2.8497x; 2.8497x over previous
"""Contrastive loss (SimCLR-style, masked-diagonal logsumexp) on 8 Trainium2
NeuronCores via Bass/Tile.

Math (matches the jax reference):
    a = anchor / ||anchor||_row ; p = positive / ||positive||_row
    F = concat([a, p])                         # [R=2B, D]
    sim = (F F^T) / T with diagonal masked
    lse_i = log(sum_{j!=i} exp(sim_ij))
    pos_i = <a_i, p_i> / T  (duplicated for both halves)
    loss = sum_i (lse_i - pos_i) * lab_i / max(sum_i lab_i, 1)

exp(sim) is symmetric, so only the upper triangle of the 16x16 grid of
512x512 blocks is computed (136 of 256 blocks): each block (I, J) yields
row-sum partials for chunk I (ACT Exp accum) and, when I != J, column-sum
partials for chunk J (ones-vector PE matmul over the exp tile).

Distribution (uniform SPMD stream, zero collectives): core c owns chunk
rows c and c+8. Step (h, g) computes block (I, (I+g) mod 16) with
I = c + 8h, g = 0..8 for h=0 and g = 0..7 for h=1 — a circular-gap
schedule that covers every unordered block pair exactly once and gives
every core the identical 17-step instruction shape. All per-core
variation lives in the host-side chunk roll (slot s holds global chunk
(c+s) mod 16), so the stationary operands sit at fixed slots 0 and 8.

Features are L2-normalized on the host, scaled by 16 and quantized to
fp8e4 (e4m3); matmuls run in DoubleRow perf mode (2 k-subtiles per
instruction, 2x PE throughput). sim = G * (1/T)/256 rides the ACT Exp
scale. The diagonal of the two diag blocks is pushed to ~-14 in sim
units by a DVE mask subtract (exp -> ~6e-7, negligible like the
reference's exp(-1e9) = 0).

The device returns raw per-step row/column-sum partials (~66 KB/core);
the host un-rolls them, adds across cores, and finishes the scalar:
loss = sum(lab*(ln(rowsum) - pos))/num_pos.
"""

import os
import numpy as np
import ml_dtypes


# ---------------------------------------------------------------- config ----
class CFG:
    B = 4096
    D = 1024
    NC = 8           # cores
    JW = 512         # chunk width (one PSUM bank of f32)
    JC = 16          # number of row/col chunks (R / JW)
    KC = 8           # k-subtiles of 128
    TEMP = 0.07
    SCALE = 16.0     # fp8 pre-scale; G = SCALE^2 * cos
    MASKSUB = 512.0  # subtracted from G on the diagonal (~2x diag value)
    MODE = "f8dr"    # 'f8dr' (fp8 DoubleRow) | 'bf16'

    @property
    def R(self):
        return 2 * self.B

    # step table: t -> (h, g, slot, isdiag); slots are per-core rolled.
    @property
    def steps(self):
        out = []
        for t in range(9):
            out.append((0, t, t, t == 0))
        for t in range(8):
            out.append((1, t, 8 + t, t == 0))
        return out

    # emission groups: lists of step indices sharing one stationary slot
    @property
    def groups(self):
        return [
            (0, [0, 1, 2]), (0, [3, 4, 5]), (0, [6, 7, 8]),
            (8, [9, 10, 11]), (8, [12, 13, 14]), (8, [15, 16]),
        ]


_BUILD_CACHE = {}


# ----------------------------------------------------------------- build ----
def build_nc(cfg: CFG):
    import concourse.bass as bass  # noqa: F401  (AP helpers live here)
    import concourse.tile as tile
    from contextlib import ExitStack
    from concourse import bacc, mybir

    f32 = mybir.dt.float32
    bf16 = mybir.dt.bfloat16
    f8 = mybir.dt.float8e4
    Act = mybir.ActivationFunctionType
    Alu = mybir.AluOpType

    JW, JC, KC = cfg.JW, cfg.JC, cfg.KC
    fp8 = cfg.MODE == "f8dr"
    mmdt = f8 if fp8 else bf16
    K2 = KC // 2 if fp8 else KC           # matmul k-steps per block
    perf = mybir.MatmulPerfMode.DoubleRow if fp8 else None
    sc = (1.0 / cfg.TEMP) / (cfg.SCALE * cfg.SCALE)
    steps = cfg.steps
    nsteps = len(steps)

    nc = bacc.Bacc("TRN2", target_bir_lowering=False, debug=False,
                   num_devices=cfg.NC)

    fch = nc.dram_tensor("fchunks", [JC, 128, KC, JW], mmdt,
                         kind="ExternalInput").ap()
    maskd = nc.dram_tensor("maskd", [128, 4, JW], f32,
                           kind="ExternalInput").ap()
    rs_out = nc.dram_tensor("rs_out", [128, nsteps, 4], f32,
                            kind="ExternalOutput").ap()
    cs_out = nc.dram_tensor("cs_out", [1, nsteps * JW], f32,
                            kind="ExternalOutput").ap()

    with tile.TileContext(nc) as tc, ExitStack() as ctx:
        const = ctx.enter_context(tc.tile_pool(name="const", bufs=1))
        esp = ctx.enter_context(tc.tile_pool(name="es", bufs=26))
        yp = ctx.enter_context(tc.tile_pool(name="y", bufs=3))
        smal = ctx.enter_context(tc.tile_pool(name="small", bufs=1))
        gp = ctx.enter_context(tc.tile_pool(name="g", bufs=6, space="PSUM"))
        csp = ctx.enter_context(tc.tile_pool(name="cs", bufs=2, space="PSUM"))

        # ---- staging -------------------------------------------------------
        # warm the Exp ACT table while DMAs stream
        dummy = smal.tile([1, 1], f32)
        nc.vector.memset(dummy[:], 0.0)
        nc.scalar.activation(dummy[:], dummy[:], Act.Exp)

        fc_sb = const.tile([128, JC, KC, JW], mmdt)
        # first group's slots split per k-pair so matmuls start early
        for s in range(3):
            for k in range(0, KC, 2):
                nc.sync.dma_start(fc_sb[:, s, k:k + 2, :], fch[s, :, k:k + 2, :])
        for s in range(3, JC):
            eng = nc.gpsimd if s % 2 else nc.sync
            eng.dma_start(fc_sb[:, s], fch[s])
        maskB = const.tile([128, 4, JW], f32)
        nc.gpsimd.dma_start(maskB[:], maskd)

        ones_bf = smal.tile([128, 1], bf16)
        nc.vector.memset(ones_bf[:], 1.0)
        rs_all = const.tile([128, nsteps, 4], f32)
        cs_sb = const.tile([1, nsteps * JW], f32)

        # ---- main loop -----------------------------------------------------
        pend = None  # (M, es_tiles) of previous group, colsums deferred

        def emit_colsums(M, es_tiles):
            for i, t in enumerate(M):
                if steps[t][3]:
                    continue
                cp = csp.tile([1, JW], f32, tag="cs")
                for mt in range(4):
                    nc.tensor.matmul(cp[:], ones_bf[:, 0:1],
                                     es_tiles[(i, mt)][:],
                                     start=(mt == 0), stop=(mt == 3))
                nc.vector.tensor_copy(cs_sb[:, t * JW:(t + 1) * JW], cp[:])

        for S, M in cfg.groups:
            es_tiles = {}
            for mt in range(4):
                Gs = [gp.tile([128, JW], f32, tag="g", name=f"g{i}")
                      for i in range(len(M))]
                for k2 in range(K2):
                    ksl = slice(2 * k2, 2 * k2 + 2) if fp8 else \
                        slice(k2, k2 + 1)
                    for i, t in enumerate(M):
                        sl = steps[t][2]
                        nc.tensor.matmul(
                            Gs[i][:],
                            fc_sb[:, S, ksl, mt * 128:(mt + 1) * 128],
                            fc_sb[:, sl, ksl, :],
                            start=(k2 == 0), stop=(k2 == K2 - 1),
                            perf_mode=perf)
                for i, t in enumerate(M):
                    es = esp.tile([128, JW], bf16, tag="es")
                    if steps[t][3]:
                        y = yp.tile([128, JW], f32, tag="y")
                        nc.vector.scalar_tensor_tensor(
                            out=y[:], in0=maskB[:, mt, :],
                            scalar=-float(cfg.MASKSUB), in1=Gs[i][:],
                            op0=Alu.mult, op1=Alu.add)
                        src = y
                    else:
                        src = Gs[i]
                    nc.scalar.activation(es[:], src[:], Act.Exp,
                                         scale=float(sc),
                                         accum_out=rs_all[:, t, mt:mt + 1])
                    es_tiles[(i, mt)] = es
            if pend is not None:
                emit_colsums(*pend)
            pend = (M, es_tiles)
        emit_colsums(*pend)

        # ---- ship raw partials --------------------------------------------
        nc.sync.dma_start(rs_out, rs_all[:])
        nc.sync.dma_start(cs_out, cs_sb[:])

    nc.finalize()
    return nc


# ------------------------------------------------------------ host side -----
def make_in_maps(cfg: CFG, feats_q: np.ndarray):
    JC, JW, KC = cfg.JC, cfg.JW, cfg.KC
    # X[j, p, k, n] = feats_q[j*JW + n, k*128 + p]
    X = feats_q.reshape(JC, JW, KC, 128).transpose(0, 3, 2, 1)
    idx = np.arange(128)
    maskB = np.zeros((128, 4, JW), np.float32)
    for mt in range(4):
        maskB[idx, mt, mt * 128 + idx] = 1.0
    in_maps = []
    for c in range(cfg.NC):
        roll = [(c + s) % JC for s in range(JC)]
        in_maps.append({
            "fchunks": np.ascontiguousarray(X[roll]),
            "maskd": maskB,
        })
    return in_maps


LAST_RESULTS = None


def kernel(anchor_features, positive_features, labels):
    global LAST_RESULTS
    from concourse.bass_utils import run_bass_kernel_spmd

    cfg = CFG()
    key = (cfg.B, cfg.D, cfg.NC, cfg.MODE)
    if key not in _BUILD_CACHE:
        _BUILD_CACHE[key] = build_nc(cfg)
    nc = _BUILD_CACHE[key]

    a = np.asarray(anchor_features, dtype=np.float32)
    p = np.asarray(positive_features, dtype=np.float32)
    lab = np.asarray(labels).astype(np.float64)
    an = a / np.linalg.norm(a, axis=1, keepdims=True)
    pn = p / np.linalg.norm(p, axis=1, keepdims=True)
    cross = np.einsum("ij,ij->i", an, pn, dtype=np.float64) / cfg.TEMP
    feats = np.concatenate([an, pn], axis=0) * cfg.SCALE
    npdt = ml_dtypes.float8_e4m3 if cfg.MODE == "f8dr" else ml_dtypes.bfloat16
    feats_q = feats.astype(npdt)

    in_maps = make_in_maps(cfg, feats_q)
    trace = bool(int(os.environ.get("KERNEL_TRACE", "0")))
    res = run_bass_kernel_spmd(nc, in_maps, list(range(cfg.NC)), trace=trace)
    LAST_RESULTS = res

    # un-roll per-core partials into the global row-sum vector
    rowsum = np.zeros(cfg.R, np.float64)
    steps = cfg.steps
    for c in range(cfg.NC):
        rs = np.asarray(res.results[c]["rs_out"], np.float64)  # [128, 17, 4]
        cs = np.asarray(res.results[c]["cs_out"],
                        np.float64).reshape(len(steps), cfg.JW)
        for t, (h, g, _slot, isdiag) in enumerate(steps):
            I = (c + 8 * h) % cfg.JC
            rowsum[I * cfg.JW:(I + 1) * cfg.JW] += rs[:, t, :].T.reshape(-1)
            if not isdiag:
                J = (I + g) % cfg.JC
                rowsum[J * cfg.JW:(J + 1) * cfg.JW] += cs[t]

    lse = np.log(rowsum)
    pos2 = np.concatenate([cross, cross])
    lab2 = np.concatenate([lab, lab])
    num_pos = lab2.sum()
    loss = (lab2 * (lse - pos2)).sum() / num_pos if num_pos > 0 else 0.0
    return np.float32(loss)


# revision 10
# speedup vs baseline: 3.7495x; 1.3158x over previous
"""Contrastive loss (SimCLR-style, masked-diagonal logsumexp) on 8 Trainium2
NeuronCores via Bass/Tile.

Math (matches the jax reference):
    a = anchor / ||anchor||_row ; p = positive / ||positive||_row
    F = concat([a, p])                         # [R=2B, D]
    sim = (F F^T) / T with diagonal masked
    lse_i = log(sum_{j!=i} exp(sim_ij))
    pos_i = <a_i, p_i> / T  (duplicated for both halves)
    loss = sum_i (lse_i - pos_i) * lab_i / max(sum_i lab_i, 1)

exp(sim) is symmetric, so only the upper triangle of the 16x16 grid of
512x512 blocks is computed (136 of 256 blocks): each block (I, J) yields
row-sum partials for chunk I (ACT Exp accum) and, when I != J, column-sum
partials for chunk J (ones-vector PE matmul over the exp tile).

Distribution (uniform SPMD stream, zero collectives): core c owns chunk
rows c and c+8. Step (h, g) computes block (I, (I+g) mod 16) with
I = c + 8h, g = 0..8 for h=0 and g = 0..7 for h=1 — a circular-gap
schedule that covers every unordered block pair exactly once and gives
every core the identical 17-step instruction shape. All per-core
variation lives in the host-side chunk roll (slot s holds global chunk
(c+s) mod 16), so the stationary operands sit at fixed slots 0 and 8.

Features are L2-normalized on the host, scaled by 16 and quantized to
fp8e4 (e4m3); matmuls run in DoubleRow perf mode (2 k-subtiles per
instruction, 2x PE throughput). sim = G * (1/T)/256 rides the ACT Exp
scale. The diagonal of the two diag blocks is pushed to ~-14 in sim
units by a DVE mask subtract (exp -> ~6e-7, negligible like the
reference's exp(-1e9) = 0). Exp tiles are written back as fp8 m-tile
pairs so each column-sum is 2 DoubleRow matmuls against a ones vector
(row sums come from the ACT accumulator, which sums in f32 pre-cast).

The device ships raw per-step row/column-sum partials (~66 KB/core)
eagerly per group; the host un-rolls them, adds across cores, and
finishes the scalar: loss = sum(lab*(ln(rowsum) - pos))/num_pos.
"""

import os
import numpy as np
import ml_dtypes


# ---------------------------------------------------------------- config ----
class CFG:
    B = 4096
    D = 1024
    NC = 8           # cores
    JW = 512         # chunk width (one PSUM bank of f32)
    JC = 16          # number of row/col chunks (R / JW)
    KC = 8           # k-subtiles of 128
    TEMP = 0.07
    SCALE = 16.0     # fp8 pre-scale; G = SCALE^2 * cos
    MASKSUB = 512.0  # subtracted from G on the diagonal (~2x diag value)
    MODE = "f8dr"    # 'f8dr' (fp8 DoubleRow) | 'bf16'

    @property
    def R(self):
        return 2 * self.B

    # step table: t -> (h, g, slot, isdiag); slots are per-core rolled.
    @property
    def steps(self):
        out = []
        for t in range(9):
            out.append((0, t, t, t == 0))
        for t in range(8):
            out.append((1, t, 8 + t, t == 0))
        return out

    # emission groups: lists of step indices sharing one stationary slot
    @property
    def groups(self):
        return [
            (0, [0, 1, 2]), (0, [3, 4, 5]), (0, [6, 7, 8]),
            (8, [9, 10, 11]), (8, [12, 13, 14]), (8, [15, 16]),
        ]


_BUILD_CACHE = {}


# ----------------------------------------------------------------- build ----
def build_nc(cfg: CFG):
    import concourse.bass as bass  # noqa: F401  (AP helpers live here)
    import concourse.tile as tile
    from contextlib import ExitStack
    from concourse import bacc, mybir

    f32 = mybir.dt.float32
    bf16 = mybir.dt.bfloat16
    f8 = mybir.dt.float8e4
    Act = mybir.ActivationFunctionType
    Alu = mybir.AluOpType

    JW, JC, KC = cfg.JW, cfg.JC, cfg.KC
    fp8 = cfg.MODE == "f8dr"
    mmdt = f8 if fp8 else bf16
    K2 = KC // 2 if fp8 else KC           # matmul k-steps per block
    perf = mybir.MatmulPerfMode.DoubleRow if fp8 else None
    sc = (1.0 / cfg.TEMP) / (cfg.SCALE * cfg.SCALE)
    steps = cfg.steps
    nsteps = len(steps)

    nc = bacc.Bacc("TRN2", target_bir_lowering=False, debug=False,
                   num_devices=cfg.NC)

    fch = nc.dram_tensor("fchunks", [JC, 128, KC, JW], mmdt,
                         kind="ExternalInput").ap()
    maskd = nc.dram_tensor("maskd", [128, 4, JW], bf16,
                           kind="ExternalInput").ap()
    rs_out = nc.dram_tensor("rs_out", [128, nsteps, 4], f32,
                            kind="ExternalOutput").ap()
    cs_out = nc.dram_tensor("cs_out", [1, nsteps * JW], f32,
                            kind="ExternalOutput").ap()

    with tile.TileContext(nc) as tc, ExitStack() as ctx:
        const = ctx.enter_context(tc.tile_pool(name="const", bufs=1))
        esp = ctx.enter_context(tc.tile_pool(name="es", bufs=14))
        yp = ctx.enter_context(tc.tile_pool(name="y", bufs=3))
        smal = ctx.enter_context(tc.tile_pool(name="small", bufs=1))
        gp = ctx.enter_context(tc.tile_pool(name="g", bufs=6, space="PSUM"))
        csp = ctx.enter_context(tc.tile_pool(name="cs", bufs=2, space="PSUM"))

        # ---- staging (all DMA issue on gpsimd: 25ns vs 565ns on sync) ------
        # warm the Exp ACT table while DMAs stream
        dummy = smal.tile([1, 1], f32)
        nc.vector.memset(dummy[:], 0.0)
        nc.scalar.activation(dummy[:], dummy[:], Act.Exp)

        fc_sb = const.tile([128, JC, KC, JW], mmdt)
        maskB = const.tile([128, 4, JW], bf16)
        # first group's slots split per k-pair so matmuls start early
        for s in range(3):
            for k in range(0, KC, 2):
                nc.gpsimd.dma_start(fc_sb[:, s, k:k + 2, :],
                                    fch[s, :, k:k + 2, :])
        for mt in range(4):
            nc.gpsimd.dma_start(maskB[:, mt, :], maskd[:, mt, :])
        for s in range(3, JC):
            nc.gpsimd.dma_start(fc_sb[:, s], fch[s])

        # [128, 2, 16]: the k-pair step must be even and 16B-aligned for
        # DoubleRow ldweights (s3_lw_dual_fp8_restrictions)
        ones_f8 = smal.tile([128, 2, 16], f8)
        nc.vector.memset(ones_f8[:], 1.0)
        ones_bf = smal.tile([128, 1], bf16)
        nc.vector.memset(ones_bf[:], 1.0)
        rs_all = const.tile([128, nsteps, 4], f32)
        cs_sb = const.tile([1, nsteps * JW], f32)

        # ---- main loop -----------------------------------------------------
        pend = None  # (M, es_tiles) of previous group, colsums deferred

        def emit_colsums(M, es_tiles):
            for i, t in enumerate(M):
                if steps[t][3]:
                    continue
                cp = csp.tile([1, JW], f32, tag="cs")
                if fp8:
                    for mtp in range(2):
                        nc.tensor.matmul(cp[:], ones_f8[:, :, 0:1],
                                         es_tiles[(i, mtp)][:],
                                         start=(mtp == 0), stop=(mtp == 1),
                                         perf_mode=perf)
                else:
                    for mtp in range(2):
                        for j in range(2):
                            nc.tensor.matmul(
                                cp[:], ones_bf[:, 0:1],
                                es_tiles[(i, mtp)][:, j, :],
                                start=(mtp == 0 and j == 0),
                                stop=(mtp == 1 and j == 1))
                nc.vector.tensor_copy(cs_sb[:, t * JW:(t + 1) * JW], cp[:])
                nc.gpsimd.dma_start(cs_out[:, t * JW:(t + 1) * JW],
                                    cs_sb[:, t * JW:(t + 1) * JW])

        for S, M in cfg.groups:
            es_tiles = {}
            # non-diag epilogues first so a late mask DMA can't stall ACT
            order = [i for i, t in enumerate(M) if not steps[t][3]] + \
                    [i for i, t in enumerate(M) if steps[t][3]]
            for mt in range(4):
                Gs = [gp.tile([128, JW], f32, tag="g", name=f"g{i}")
                      for i in range(len(M))]
                for k2 in range(K2):
                    ksl = slice(2 * k2, 2 * k2 + 2) if fp8 else \
                        slice(k2, k2 + 1)
                    for i, t in enumerate(M):
                        sl = steps[t][2]
                        nc.tensor.matmul(
                            Gs[i][:],
                            fc_sb[:, S, ksl, mt * 128:(mt + 1) * 128],
                            fc_sb[:, sl, ksl, :],
                            start=(k2 == 0), stop=(k2 == K2 - 1),
                            perf_mode=perf)
                for i in order:
                    t = M[i]
                    if mt % 2 == 0:
                        es_tiles[(i, mt // 2)] = esp.tile(
                            [128, 2, JW], mmdt, tag="es", name="es")
                    es = es_tiles[(i, mt // 2)]
                    if steps[t][3]:
                        y = yp.tile([128, JW], f32, tag="y")
                        nc.vector.scalar_tensor_tensor(
                            out=y[:], in0=maskB[:, mt, :],
                            scalar=-float(cfg.MASKSUB), in1=Gs[i][:],
                            op0=Alu.mult, op1=Alu.add)
                        src = y[:]
                    else:
                        src = Gs[i][:]
                    nc.scalar.activation(es[:, mt % 2, :], src, Act.Exp,
                                         scale=float(sc),
                                         accum_out=rs_all[:, t, mt:mt + 1])
            if pend is not None:
                emit_colsums(*pend)
            # ship this group's row-sum partials
            t0, t1 = M[0], M[-1] + 1
            nc.gpsimd.dma_start(rs_out[:, t0:t1, :], rs_all[:, t0:t1, :])
            pend = (M, es_tiles)
        emit_colsums(*pend)

    nc.finalize()
    return nc


# ------------------------------------------------------------ host side -----
def make_in_maps(cfg: CFG, feats_q: np.ndarray):
    JC, JW, KC = cfg.JC, cfg.JW, cfg.KC
    # X[j, p, k, n] = feats_q[j*JW + n, k*128 + p]
    X = feats_q.reshape(JC, JW, KC, 128).transpose(0, 3, 2, 1)
    idx = np.arange(128)
    maskB = np.zeros((128, 4, JW), ml_dtypes.bfloat16)
    for mt in range(4):
        maskB[idx, mt, mt * 128 + idx] = 1.0
    in_maps = []
    for c in range(cfg.NC):
        roll = [(c + s) % JC for s in range(JC)]
        in_maps.append({
            "fchunks": np.ascontiguousarray(X[roll]),
            "maskd": maskB,
        })
    return in_maps


LAST_RESULTS = None


def kernel(anchor_features, positive_features, labels):
    global LAST_RESULTS
    from concourse.bass_utils import run_bass_kernel_spmd

    cfg = CFG()
    key = (cfg.B, cfg.D, cfg.NC, cfg.MODE)
    if key not in _BUILD_CACHE:
        _BUILD_CACHE[key] = build_nc(cfg)
    nc = _BUILD_CACHE[key]

    a = np.asarray(anchor_features, dtype=np.float32)
    p = np.asarray(positive_features, dtype=np.float32)
    lab = np.asarray(labels).astype(np.float64)
    an = a / np.linalg.norm(a, axis=1, keepdims=True)
    pn = p / np.linalg.norm(p, axis=1, keepdims=True)
    cross = np.einsum("ij,ij->i", an, pn, dtype=np.float64) / cfg.TEMP
    feats = np.concatenate([an, pn], axis=0) * cfg.SCALE
    npdt = ml_dtypes.float8_e4m3 if cfg.MODE == "f8dr" else ml_dtypes.bfloat16
    feats_q = feats.astype(npdt)

    in_maps = make_in_maps(cfg, feats_q)
    trace = bool(int(os.environ.get("KERNEL_TRACE", "0")))
    res = run_bass_kernel_spmd(nc, in_maps, list(range(cfg.NC)), trace=trace)
    LAST_RESULTS = res

    # un-roll per-core partials into the global row-sum vector
    rowsum = np.zeros(cfg.R, np.float64)
    steps = cfg.steps
    for c in range(cfg.NC):
        rs = np.asarray(res.results[c]["rs_out"], np.float64)  # [128, 17, 4]
        cs = np.asarray(res.results[c]["cs_out"],
                        np.float64).reshape(len(steps), cfg.JW)
        for t, (h, g, _slot, isdiag) in enumerate(steps):
            I = (c + 8 * h) % cfg.JC
            rowsum[I * cfg.JW:(I + 1) * cfg.JW] += rs[:, t, :].T.reshape(-1)
            if not isdiag:
                J = (I + g) % cfg.JC
                rowsum[J * cfg.JW:(J + 1) * cfg.JW] += cs[t]

    lse = np.log(rowsum)
    pos2 = np.concatenate([cross, cross])
    lab2 = np.concatenate([lab, lab])
    num_pos = lab2.sum()
    loss = (lab2 * (lse - pos2)).sum() / num_pos if num_pos > 0 else 0.0
    return np.float32(loss)
